# revision 1
# baseline (speedup 1.0000x reference)
"""Trainium2 Bass kernel for a pre-norm transformer block (attention + MLP).

Shapes: x [4, 1024, 1024], H=16 heads, Dh=64, MLP hidden 4096, f32.

Strategy (8 NeuronCores, no collectives):
  - Token-sharded: core c handles batch row b=c//2, query tokens
    [off, off+512), off=(c%2)*512. Both cores of a pair redundantly
    compute K/V over the full 1024-token row (no cross-core comms);
    everything else is perfectly sharded.
  - Activations flow in transposed layout [feature(partition), token(free)];
    weights are transposed on the host so every matmul streams naturally.
  - LayerNorm gains are folded into the following weight matrix on the host
    (biases asserted zero); LN1 is fused algebraically into the QKV
    evictions:  ln(x) @ W'^T = rstd*(x @ W'^T) + (-mu*rstd)*rowsum(W').
  - Per-token LN stats via ones-vector matmuls on the PE (partition-dim
    reductions), broadcast back with K=1 matmuls.
  - Softmax denominator folded into the attention A@V matmul via an
    appended ones-column on V (row 64 of the PSUM output is sum(exp)).
  - Matmuls in float32r (TF32-class, full PE rate); the attention path
    (S^T, exp(S)@[V|1], proj) runs in bf16.
"""

import sys

try:
    import concourse  # noqa: F401
except ImportError:  # pragma: no cover
    sys.path.insert(0, "/opt/trn_rl_repo")

import ml_dtypes
import numpy as np

import concourse.bass as bass  # noqa: F401
import concourse.tile as tile
from concourse import bacc, bass_utils, mybir

F32 = mybir.dt.float32
F32R = mybir.dt.float32r
BF16 = mybir.dt.bfloat16
AF = mybir.ActivationFunctionType
OP = mybir.AluOpType

P = 128
C = 1024
N = 1024
B = 4
H = 16
DH = 64
DFF = 4096
TOK = 512          # per-core query-token block
NCT = C // P       # 8 c-tiles
NFT = DFF // P     # 32 d'-tiles
EPS = 1e-5
SCALE = DH ** -0.5

_CACHE = {}


def build():
    nc = bacc.Bacc(
        "TRN2",
        target_bir_lowering=False,
        debug=False,
        enable_asserts=False,
        num_devices=8,
    )

    def din(name, shape, dt=F32R):
        return nc.dram_tensor(name, shape, dt, kind="ExternalInput").ap()

    xrow = din("xrow", [C, N], BF16)      # x[b].T (bf16)
    xq = din("xq", [C, TOK])              # query-token slice of x[b].T
    wq = din("wq", [C, C])                # (q rows of qkv_w * ln1_g).T
    wkv = din("wkv", [C, 2 * C], BF16)    # (k,v rows, folded).T (bf16)
    wproj = din("wproj", [C, C], BF16)    # proj_w.T (bf16)
    wfc1 = din("wfc1", [C, DFF], BF16)    # (fc1_w * ln2_g).T (bf16)
    wfc2 = din("wfc2", [DFF, C], BF16)    # (fc2_w * lnh_g).T (bf16)
    wqs = din("wqs", [P, 3 * NCT], F32)   # per-col rowsums of folded qkv_w
    wvs = din("wvs", [1, C], F32R)        # rowsums for V cols (row layout)
    pb = din("pb", [P, NCT], F32)
    f1b = din("f1b", [P, NFT], F32)
    f2b = din("f2b", [P, NCT], F32)
    ones1_d = din("ones1", [1, P])        # K=1 broadcast lhsT
    ones128_d = din("ones128", [P, 1])    # partition-sum lhsT

    outT = nc.dram_tensor("outT", [C, TOK], F32, kind="ExternalOutput").ap()

    with tile.TileContext(nc) as tc:
        const = tc.alloc_tile_pool(name="const", bufs=1)
        big = tc.alloc_tile_pool(name="big", bufs=1)
        tmp = tc.alloc_tile_pool(name="tmp", bufs=2)
        misc = tc.alloc_tile_pool(name="misc", bufs=1)
        wpool = tc.alloc_tile_pool(name="w", bufs=10)

        # --- constants ---
        ones1 = const.tile([1, P], F32R)
        nc.sync.dma_start(ones1[:], ones1_d[:])
        ones1h = const.tile([DH + 1, P], F32R)   # ones row AT partition 64
        nc.sync.dma_start(ones1h[DH:DH + 1, :], ones1_d[:])
        ones128 = const.tile([P, 1], F32R)
        nc.sync.dma_start(ones128[:], ones128_d[:])
        ones128b = const.tile([P, 1], BF16)
        nc.vector.memset(ones128b[:], 1.0)
        eps = const.tile([1, 1], F32)
        nc.vector.memset(eps[:], EPS)
        gb = {}
        for nm, ap_, w in (("wqs", wqs, 3 * NCT), ("pb", pb, NCT),
                           ("f1b", f1b, NFT), ("f2b", f2b, NCT)):
            t = const.tile([P, w], F32, name=nm, tag=nm)
            nc.sync.dma_start(t[:], ap_[:])
            gb[nm] = t
        wvs_s = const.tile([1, C], F32R)
        nc.sync.dma_start(wvs_s[:], wvs[:])

        def ln_finish(ps_s, ps_q, n_elems):
            inv = 1.0 / n_elems
            mu = misc.tile([1, TOK], F32R, tag="ln_mu", bufs=2, name="mu_f")
            nc.vector.tensor_scalar_mul(mu[:], ps_s[:], inv)
            ex2 = misc.tile([1, TOK], F32, tag="ln_ex2", bufs=2, name="ex2_f")
            nc.vector.tensor_scalar_mul(ex2[:], ps_q[:], inv)
            mu2 = misc.tile([1, TOK], F32, tag="ln_mu2", bufs=2, name="mu2_f")
            nc.vector.tensor_mul(mu2[:], mu[:], mu[:])
            nc.vector.tensor_sub(ex2[:], ex2[:], mu2[:])
            nc.scalar.activation(ex2[:], ex2[:], AF.Ln, bias=eps[:])
            rstd = misc.tile([1, TOK], F32R, tag="ln_rstd", bufs=2, name="rstd_f")
            nc.scalar.activation(rstd[:], ex2[:], AF.Exp, scale=-0.5)
            return mu, rstd

        def ln_stats(stat_ps, src_tiles, n_ct, ones_lhs):
            """Returns (mu, rstd) [1, TOK] SBUF rows (f32r)."""
            ps_s = stat_ps.tile([1, TOK], F32, tag="ln_s")
            ps_q = stat_ps.tile([1, TOK], F32, tag="ln_q")
            for ci in range(n_ct):
                s = src_tiles(ci)
                sq = tmp.tile([P, TOK], F32R, tag="ln_sq")
                nc.scalar.activation(sq[:], s, AF.Square)
                nc.tensor.matmul(ps_s[:], ones_lhs[:], s,
                                 start=(ci == 0), stop=(ci == n_ct - 1))
                nc.tensor.matmul(ps_q[:], ones128[:], sq[:],
                                 start=(ci == 0), stop=(ci == n_ct - 1))
            inv = 1.0 / (n_ct * P)
            mu = misc.tile([1, TOK], F32R, tag="ln_mu", bufs=2)
            nc.vector.tensor_scalar_mul(mu[:], ps_s[:], inv)
            ex2 = misc.tile([1, TOK], F32, tag="ln_ex2", bufs=2)
            nc.vector.tensor_scalar_mul(ex2[:], ps_q[:], inv)
            mu2 = misc.tile([1, TOK], F32, tag="ln_mu2", bufs=2)
            nc.vector.tensor_mul(mu2[:], mu[:], mu[:])
            nc.vector.tensor_sub(ex2[:], ex2[:], mu2[:])      # var, in place
            nc.scalar.activation(ex2[:], ex2[:], AF.Ln, bias=eps[:])
            rstd = misc.tile([1, TOK], F32R, tag="ln_rstd", bufs=2)
            nc.scalar.activation(rstd[:], ex2[:], AF.Exp, scale=-0.5)
            return mu, rstd

        # --- load x ---
        xr = big.tile([P, NCT, N], BF16, tag="A")      # x[b].T tiled (bf16)
        xrs = xrow.rearrange("(i p) t -> p i t", p=P)
        for ci in range(NCT):
            nc.sync.dma_start(xr[:, ci, :], xrs[:, ci, :])
        xqt = big.tile([P, NCT, TOK], F32R, tag="D")   # query slice (residual)
        xqs = xq.rearrange("(i p) t -> p i t", p=P)
        for ci in range(NCT):
            nc.sync.dma_start(xqt[:, ci, :], xqs[:, ci, :])

        # --- LN1 stats for row blocks and query block; broadcast rstd/-mu*rstd ---
        ps_stat = tc.alloc_tile_pool(name="ps_stat1", bufs=2, space="PSUM")
        ps_bc = tc.alloc_tile_pool(name="ps_bc1", bufs=1, space="PSUM")
        ps_tp = tc.alloc_tile_pool(name="ps_tp", bufs=2, space="PSUM")
        rstd_sb = []   # [128, TOK] f32 per block (0,1 = row blocks, 2 = q)
        nmr_sb = []
        rstdT = misc.tile([P, NCT], F32, tag="rstdT")   # column form per tok-tile
        nmrT = misc.tile([P, NCT], F32, tag="nmrT")
        for blk in range(3):
            if blk < 2:
                sl = slice(blk * TOK, (blk + 1) * TOK)
                mu, rstd = ln_stats(ps_stat, lambda ci: xr[:, ci, sl], NCT, ones128b)
            else:
                mu, rstd = ln_stats(ps_stat, lambda ci: xqt[:, ci, :], NCT, ones128)
            nmr = misc.tile([1, TOK], F32R, tag="ln_nmr", bufs=2)
            nc.vector.tensor_mul(nmr[:], mu[:], rstd[:])
            nc.vector.tensor_scalar_mul(nmr[:], nmr[:], -1.0)
            bc_r = ps_bc.tile([P, TOK], F32, tag="bc_r")
            nc.tensor.matmul(bc_r[:], ones1[:], rstd[:], start=True, stop=True)
            bc_n = ps_bc.tile([P, TOK], F32, tag="bc_n")
            nc.tensor.matmul(bc_n[:], ones1[:], nmr[:], start=True, stop=True)
            r_sb = misc.tile([P, TOK], BF16, tag="lnsb", bufs=6, name=f"rsb{blk}")
            nc.vector.tensor_copy(r_sb[:], bc_r[:])
            n_sb = misc.tile([P, TOK], BF16, tag="lnsb", bufs=6, name=f"nsb{blk}")
            nc.vector.tensor_copy(n_sb[:], bc_n[:])
            rstd_sb.append(r_sb)
            nmr_sb.append(n_sb)
            if blk < 2:
                # transpose rstd/nmr rows into per-token-tile columns (for V)
                for sub in range(4):
                    r = blk * 4 + sub
                    cs = slice(sub * P, (sub + 1) * P)
                    pt = ps_tp.tile([P, 1], F32, tag="tp", name=f"tp{r}")
                    nc.tensor.matmul(pt[:], rstd[0:1, cs].bitcast(F32),
                                     ones1[0:1, 0:1].bitcast(F32),
                                     start=True, stop=True)
                    nc.vector.tensor_copy(rstdT[:, r:r + 1], pt[:])
                    pt2 = ps_tp.tile([P, 1], F32, tag="tp", name=f"tp2_{r}")
                    nc.tensor.matmul(pt2[:], nmr[0:1, cs].bitcast(F32),
                                     ones1[0:1, 0:1].bitcast(F32),
                                     start=True, stop=True)
                    nc.vector.tensor_copy(nmrT[:, r:r + 1], pt2[:])
        # broadcast V-column rowsums to all partitions (once)
        wvs_b = misc.tile([P, C], BF16, tag="wvs_b")
        for g in range(2):
            bc = ps_bc.tile([P, TOK], F32, tag="bc_r", name=f"bcv{g}")
            nc.tensor.matmul(bc[:], ones1[:], wvs_s[0:1, g * TOK:(g + 1) * TOK],
                             start=True, stop=True)
            nc.vector.tensor_copy(wvs_b[:, g * TOK:(g + 1) * TOK], bc[:])
        ps_tp.release()
        ps_bc.release()
        ps_stat.release()

        # --- QKV with fused LN1 (transposed QT/KT bf16, natural V bf16 + ones) ---
        KT = big.tile([P, NCT, N], BF16, tag="B")
        QT = big.tile([P, NCT, TOK], BF16, tag="F")
        V = big.tile([P, NCT, H, DH + 1], BF16, tag="V")
        for r in range(NCT):
            nc.vector.memset(V[:, r, :, DH:DH + 1], 1.0)

        ps_acc = tc.alloc_tile_pool(name="ps_qkv", bufs=2, space="PSUM")
        ps_s = tc.alloc_tile_pool(name="ps_s", bufs=3, space="PSUM")

        def qk_group(g):
            wt = []
            for ci in range(NCT):
                if g < 2:
                    w = wpool.tile([P, 512], F32R, tag="w", name=f"wq{g}_{ci}")
                    nc.sync.dma_start(w[:], wq[ci * P:(ci + 1) * P, g * 512:(g + 1) * 512])
                else:
                    w = wpool.tile([P, 512], BF16, tag="wb", name=f"wk{g}_{ci}")
                    nc.sync.dma_start(w[:], wkv[ci * P:(ci + 1) * P, (g - 2) * 512:(g - 1) * 512])
                wt.append(w)
            if g < 2:  # Q -> QT (query block, fused LN)
                for jt in range(4):
                    jj = g * 4 + jt
                    ps = ps_acc.tile([P, TOK], F32, tag="acc")
                    for ci in range(NCT):
                        nc.tensor.matmul(ps[:], wt[ci][:, jt * P:(jt + 1) * P],
                                         xqt[:, ci, :], start=(ci == 0), stop=(ci == NCT - 1))
                    t = tmp.tile([P, TOK], BF16, tag="ev", bufs=3)
                    nc.vector.tensor_mul(t[:], ps[:], rstd_sb[2][:])
                    nc.vector.scalar_tensor_tensor(
                        QT[:, jj, :], nmr_sb[2][:], gb["wqs"][:, jj:jj + 1], t[:],
                        op0=OP.mult, op1=OP.add)
            else:  # K -> KT
                for jt in range(4):
                    jj = (g - 2) * 4 + jt
                    for blk in range(2):
                        ps = ps_acc.tile([P, TOK], F32, tag="acc")
                        for ci in range(NCT):
                            nc.tensor.matmul(ps[:], wt[ci][:, jt * P:(jt + 1) * P],
                                             xr[:, ci, blk * TOK:(blk + 1) * TOK],
                                             start=(ci == 0), stop=(ci == NCT - 1))
                        t = tmp.tile([P, TOK], BF16, tag="ev", bufs=3)
                        nc.vector.tensor_mul(t[:], ps[:], rstd_sb[blk][:])
                        nc.vector.scalar_tensor_tensor(
                            KT[:, jj, blk * TOK:(blk + 1) * TOK],
                            nmr_sb[blk][:], gb["wqs"][:, NCT + jj:NCT + jj + 1], t[:],
                            op0=OP.mult, op1=OP.add)

        def v_group(g):  # g in (4, 5)
            wt = []
            for ci in range(NCT):
                w = wpool.tile([P, 512], BF16, tag="wb", name=f"wv{g}_{ci}")
                nc.sync.dma_start(w[:], wkv[ci * P:(ci + 1) * P, (g - 2) * 512:(g - 1) * 512])
                wt.append(w)
            h0 = 8 * (g - 4)
            dsl = slice((g - 4) * TOK, (g - 4 + 1) * TOK)
            for r in range(NCT):
                ps = ps_acc.tile([P, TOK], F32, tag="acc")
                for ci in range(NCT):
                    nc.tensor.matmul(ps[:], xr[:, ci, r * P:(r + 1) * P],
                                     wt[ci][:], start=(ci == 0), stop=(ci == NCT - 1))
                t = tmp.tile([P, TOK], BF16, tag="ev", bufs=3)
                nc.vector.tensor_scalar_mul(t[:], wvs_b[:, dsl], nmrT[:, r:r + 1])
                nc.vector.scalar_tensor_tensor(
                    V[:, r, h0:h0 + 8, 0:DH],
                    ps[:].rearrange("p (h d) -> p h d", h=8),
                    rstdT[:, r:r + 1],
                    t[:].rearrange("p (h d) -> p h d", h=8),
                    op0=OP.mult, op1=OP.add)

        Es = {}

        def s_exp(jj):
            E_l = []
            for kt in range(NCT):
                E_t = big.tile([P, 2 * TOK], BF16, tag="E", bufs=16,
                               name=f"E{jj}_{kt}")
                E_l.append(E_t)
                ks = slice(kt * P, (kt + 1) * P)
                psa = ps_s.tile([P, TOK], F32, tag="S", name=f"Sa{jj}_{kt}")
                nc.tensor.matmul(psa[:], KT[0:64, jj, ks], QT[0:64, jj, :],
                                 start=True, stop=True, tile_position=(0, 0))
                nc.scalar.activation(E_t[:, 0:TOK], psa[:], AF.Exp, scale=SCALE)
                psb = ps_s.tile([P, TOK], F32, tag="S", name=f"Sb{jj}_{kt}")
                nc.tensor.matmul(psb[:], KT[64:128, jj, ks], QT[64:128, jj, :],
                                 start=True, stop=True, tile_position=(64, 0))
                nc.scalar.activation(E_t[:, TOK:2 * TOK], psb[:], AF.Exp, scale=SCALE)
            Es[jj] = E_l

        def av(jj):
            E_l = Es.pop(jj)
            for half in range(2):
                h = 2 * jj + half
                es = slice(half * TOK, (half + 1) * TOK)
                po = ps_o.tile([P, TOK], F32, tag="O")
                for kt in range(NCT):
                    nc.tensor.matmul(po[0:DH + 1, :], V[:, kt, h, :], E_l[kt][:, es],
                                     start=(kt == 0), stop=(kt == NCT - 1))
                rec = misc.tile([DH + 1, TOK], F32R, tag="rec", bufs=2)
                with nc.allow_low_precision(reason="softmax denom to f32r bcast"):
                    nc.vector.reciprocal(rec[DH:DH + 1, :], po[DH:DH + 1, :])
                pl = ps_o.tile([P, TOK], F32, tag="lbc", bufs=1, name=f"lbc{jj}_{half}")
                nc.tensor.matmul(pl[0:DH, :], ones1h[DH:DH + 1, 0:DH],
                                 rec[DH:DH + 1, :], start=True, stop=True)
                pls = misc.tile([DH, TOK], BF16, tag="pls", bufs=2)
                nc.vector.tensor_copy(pls[:], pl[0:DH, :])
                if half == 0:
                    nc.vector.tensor_mul(OT[0:DH, jj, :], po[0:DH, :], pls[:])
                else:
                    sh = misc.tile([DH, TOK], BF16, tag="shift", bufs=2)
                    nc.vector.tensor_mul(sh[:], po[0:DH, :], pls[:])
                    nc.gpsimd.dma_start(OT[DH:P, jj, :], sh[:])

        OT = big.tile([P, NCT, TOK], BF16, tag="C")
        ps_o = tc.alloc_tile_pool(name="ps_o", bufs=2, space="PSUM")

        for g in range(4):
            qk_group(g)
        s_exp(0)
        s_exp(1)
        v_group(4)
        v_group(5)
        for jj in range(NCT):
            av(jj)
            if jj + 2 < NCT:
                s_exp(jj + 2)
        for p_ in (ps_o, ps_s, ps_acc):
            p_.release()

        # --- output projection (bf16) + residual -> x2 (LN2 stats inline) ---
        x2 = big.tile([P, NCT, TOK], F32R, tag="B")   # reuses KT slot
        ps_stat = tc.alloc_tile_pool(name="ps_stat2", bufs=1, space="PSUM")
        st_s = ps_stat.tile([1, TOK], F32, tag="ln_s")
        st_q = ps_stat.tile([1, TOK], F32, tag="ln_q")
        ps_acc = tc.alloc_tile_pool(name="ps_proj", bufs=6, space="PSUM")
        for ig in range(2):
            wt = []
            for ci in range(NCT):
                w = wpool.tile([P, 512], BF16, tag="wb")
                nc.sync.dma_start(w[:], wproj[ci * P:(ci + 1) * P, ig * 512:(ig + 1) * 512])
                wt.append(w)
            for i4 in range(4):
                i = ig * 4 + i4
                ps = ps_acc.tile([P, TOK], F32, tag="acc")
                for ci in range(NCT):
                    nc.tensor.matmul(ps[:], wt[ci][:, i4 * P:(i4 + 1) * P],
                                     OT[:, ci, :], start=(ci == 0), stop=(ci == NCT - 1))
                nc.vector.scalar_tensor_tensor(
                    x2[:, i, :], ps[:], gb["pb"][:, i:i + 1], xqt[:, i, :],
                    op0=OP.add, op1=OP.add)
                sq = tmp.tile([P, TOK], F32R, tag="ln_sq")
                nc.scalar.activation(sq[:], x2[:, i, :], AF.Square)
                nc.tensor.matmul(st_s[:], ones128[:], x2[:, i, :],
                                 start=(i == 0), stop=(i == NCT - 1))
                nc.tensor.matmul(st_q[:], ones128[:], sq[:],
                                 start=(i == 0), stop=(i == NCT - 1))
        ps_acc.release()

        # --- LN2 (bare; ln2_g folded into wfc1) ---
        x2n = big.tile([P, NCT, TOK], BF16, tag="F")   # reuses QT slot
        ps_bc = tc.alloc_tile_pool(name="ps_bc2", bufs=1, space="PSUM")
        mu, rstd = ln_finish(st_s, st_q, NCT * P)
        mu_b = ps_bc.tile([P, TOK], F32, tag="mu_b")
        nc.tensor.matmul(mu_b[:], ones1[:], mu[:], start=True, stop=True)
        rstd_b = ps_bc.tile([P, TOK], F32, tag="rstd_b")
        nc.tensor.matmul(rstd_b[:], ones1[:], rstd[:], start=True, stop=True)
        rb_s = misc.tile([P, TOK], BF16, tag="lnsb", bufs=6, name="rb_s2")
        nc.vector.tensor_copy(rb_s[:], rstd_b[:])
        for ci in range(NCT):
            nc.vector.tensor_sub(x2n[:, ci, :], x2[:, ci, :], mu_b[:])
            nc.vector.tensor_mul(x2n[:, ci, :], x2n[:, ci, :], rb_s[:])
        ps_bc.release()
        ps_stat.release()  # LN2 stat banks free before fc1 needs PSUM

        # --- fc1 + gelu -> U (bf16, split over the V and E slots) ---
        U0 = big.tile([P, NFT // 2, TOK], BF16, tag="V")   # reuses V slot
        U1 = big.tile([P, NFT // 2, TOK], BF16, tag="D")  # reuses xqt slot

        def u_tile(i):
            return (U0 if i < NFT // 2 else U1)[:, i % (NFT // 2), :]

        ps_stath = tc.alloc_tile_pool(name="ps_stath", bufs=1, space="PSUM")
        sh_s = ps_stath.tile([1, TOK], F32, tag="lnh_s")
        sh_q = ps_stath.tile([1, TOK], F32, tag="lnh_q")
        ps_acc = tc.alloc_tile_pool(name="ps_fc1", bufs=6, space="PSUM")
        for ig in range(8):
            wt = []
            for ci in range(NCT):
                w = wpool.tile([P, 512], BF16, tag="wb")
                nc.sync.dma_start(w[:], wfc1[ci * P:(ci + 1) * P, ig * 512:(ig + 1) * 512])
                wt.append(w)
            for i4 in range(4):
                i = ig * 4 + i4
                ps = ps_acc.tile([P, TOK], F32, tag="acc")
                for ci in range(NCT):
                    nc.tensor.matmul(ps[:], wt[ci][:, i4 * P:(i4 + 1) * P],
                                     x2n[:, ci, :], start=(ci == 0), stop=(ci == NCT - 1))
                nc.scalar.activation(u_tile(i), ps[:], AF.Gelu,
                                     bias=gb["f1b"][:, i:i + 1])
                sq = tmp.tile([P, TOK], F32R, tag="ln_sq")
                nc.scalar.activation(sq[:], u_tile(i), AF.Square)
                nc.tensor.matmul(sh_s[:], ones128b[:], u_tile(i),
                                 start=(i == 0), stop=(i == NFT - 1))
                nc.tensor.matmul(sh_q[:], ones128[:], sq[:],
                                 start=(i == 0), stop=(i == NFT - 1))
        ps_acc.release()

        # --- LNh stats (bare; lnh_g folded into wfc2) ---
        ps_bc = tc.alloc_tile_pool(name="ps_bch", bufs=1, space="PSUM")
        mu, rstd = ln_finish(sh_s, sh_q, NFT * P)
        mu_b = ps_bc.tile([P, TOK], F32, tag="mu_bh")
        nc.tensor.matmul(mu_b[:], ones1[:], mu[:], start=True, stop=True)
        rstd_b = ps_bc.tile([P, TOK], F32, tag="rstd_bh")
        nc.tensor.matmul(rstd_b[:], ones1[:], rstd[:], start=True, stop=True)
        mu_s = misc.tile([P, TOK], BF16, tag="lnsb", bufs=6, name="mu_sh")
        nc.vector.tensor_copy(mu_s[:], mu_b[:])
        rstd_s = misc.tile([P, TOK], BF16, tag="lnsb", bufs=6, name="rstd_sh")
        nc.vector.tensor_copy(rstd_s[:], rstd_b[:])
        ps_bc.release()
        ps_stath.release()

        # --- fc2 (streamed over d' with 8 resident accumulators) + residual ---
        ps_fc2 = tc.alloc_tile_pool(name="ps_fc2", bufs=1, space="PSUM")
        fps = [ps_fc2.tile([P, TOK], F32, tag=f"fc2_{j}", name=f"fc2_{j}")
               for j in range(NCT)]
        for i in range(NFT):
            un = tmp.tile([P, TOK], BF16, tag="un")
            nc.vector.tensor_sub(un[:], u_tile(i), mu_s[:])
            nc.vector.tensor_mul(un[:], un[:], rstd_s[:])
            wa = wpool.tile([P, 512], BF16, tag="wb")
            nc.sync.dma_start(wa[:], wfc2[i * P:(i + 1) * P, 0:512])
            wb = wpool.tile([P, 512], BF16, tag="wb")
            nc.sync.dma_start(wb[:], wfc2[i * P:(i + 1) * P, 512:1024])
            for j in range(NCT):
                w = wa if j < 4 else wb
                nc.tensor.matmul(fps[j][:], w[:, (j % 4) * P:(j % 4 + 1) * P], un[:],
                                 start=(i == 0), stop=(i == NFT - 1))
        for j in range(NCT):
            ot = tmp.tile([P, TOK], F32, tag="out")
            nc.vector.scalar_tensor_tensor(
                ot[:], fps[j][:], gb["f2b"][:, j:j + 1], x2[:, j, :],
                op0=OP.add, op1=OP.add)
            nc.sync.dma_start(outT[j * P:(j + 1) * P, :], ot[:])
        ps_fc2.release()

        for p_ in (wpool, misc, tmp, big, const):
            p_.release()

    nc.compile()
    return nc


def _prep_inputs(inputs):
    """Host-side transposes/folds/slices -> per-core in_maps."""
    f = lambda a: np.asarray(a, dtype=np.float32)
    x = f(inputs["x"])
    xT = np.ascontiguousarray(x.transpose(0, 2, 1))          # [B, C, N]

    g1, b1 = f(inputs["ln1_g"]), f(inputs["ln1_b"])
    g2, b2 = f(inputs["ln2_g"]), f(inputs["ln2_b"])
    ghv, bhv = f(inputs["lnh_g"]), f(inputs["lnh_b"])
    for nm, bb in (("ln1_b", b1), ("ln2_b", b2), ("lnh_b", bhv)):
        if np.abs(bb).max() != 0.0:
            raise NotImplementedError(f"{nm} != 0 not supported by this kernel")

    qkv_f = f(inputs["qkv_w"]) * g1[None, :]      # fold ln1_g
    fc1_f = f(inputs["fc1_w"]) * g2[None, :]      # fold ln2_g
    fc2_f = f(inputs["fc2_w"]) * ghv[None, :]     # fold lnh_g
    qs = qkv_f.sum(axis=1)                        # [3072] rowsums

    common = {
        "wq": np.ascontiguousarray(qkv_f[:C].T),
        "wkv": np.ascontiguousarray(qkv_f[C:].T.astype(ml_dtypes.bfloat16)),
        "wproj": np.ascontiguousarray(f(inputs["proj_w"]).T.astype(ml_dtypes.bfloat16)),
        "wfc1": np.ascontiguousarray(fc1_f.T.astype(ml_dtypes.bfloat16)),
        "wfc2": np.ascontiguousarray(fc2_f.T.astype(ml_dtypes.bfloat16)),
        "wqs": np.ascontiguousarray(qs.reshape(3 * NCT, P).T),
        "wvs": np.ascontiguousarray(qs[2 * C:].reshape(1, C)),
        "pb": np.ascontiguousarray(f(inputs["proj_b"]).reshape(NCT, P).T),
        "f1b": np.ascontiguousarray(f(inputs["fc1_b"]).reshape(NFT, P).T),
        "f2b": np.ascontiguousarray(f(inputs["fc2_b"]).reshape(NCT, P).T),
        "ones1": np.ones((1, P), np.float32),
        "ones128": np.ones((P, 1), np.float32),
    }
    in_maps = []
    for c in range(8):
        b, off = c // 2, (c % 2) * TOK
        m = dict(common)
        m["xrow"] = np.ascontiguousarray(xT[b].astype(ml_dtypes.bfloat16))
        m["xq"] = np.ascontiguousarray(xT[b][:, off:off + TOK])
        in_maps.append(m)
    return in_maps


def _assemble(results):
    out = np.empty((B, N, C), np.float32)
    for c in range(8):
        b, off = c // 2, (c % 2) * TOK
        out[b, off:off + TOK, :] = results[c]["outT"].T
    return out


def kernel(**inputs) -> np.ndarray:
    nc = _CACHE.get("nc")
    if nc is None:
        nc = build()
        _CACHE["nc"] = nc
    in_maps = _prep_inputs(inputs)
    res = bass_utils.run_bass_kernel_spmd(nc, in_maps, core_ids=list(range(8)))
    return _assemble(res.results)



# revision 25
# speedup vs baseline: 1.1816x; 1.1816x over previous
"""Trainium2 Bass kernel for a pre-norm transformer block (attention + MLP).

Shapes: x [4, 1024, 1024], H=16 heads, Dh=64, MLP hidden 4096, f32.

Strategy (8 NeuronCores, no collectives):
  - Token-sharded: core c handles batch row b=c//2, query tokens
    [off, off+512), off=(c%2)*512. The host ROTATES each core's row so its
    query window is always columns 0:512 (k-token permutation is invariant
    under softmax+AV). Both cores of a pair redundantly compute K/V over the
    full 1024-token row.
  - Activations flow in transposed layout [feature(partition), token(free)];
    weights are transposed on the host; everything bf16 on the PE.
  - All three LayerNorms are folded algebraically into the following matmul's
    eviction: ln(x) @ W'^T = rstd*(x @ W'^T) + (-mu*rstd)*colsum(W'), with
    ln gains folded into W' on the host (LN biases asserted zero).
  - LN stats per token via N=1 matmuls (data chunk as stationary operand,
    ones column as moving operand) -> per-token stat columns; rows recovered
    with tiny transposes against an identity and broadcast with K=1 matmuls.
  - Softmax denominator folded into the attention A@V matmul via an appended
    ones-column on V; A@V computed transposed (E chunks stationary, V moving,
    N=65) then flipped back with paired-head 128x128 PE transposes.
  - fc2 runs split in two halves: the first four output tiles accumulate
    interleaved with fc1 (lagged one hidden tile), the rest stream after.
"""

import sys

try:
    import concourse  # noqa: F401
except ImportError:  # pragma: no cover
    sys.path.insert(0, "/opt/trn_rl_repo")

import ml_dtypes
import numpy as np

import concourse.bass as bass  # noqa: F401
import concourse.tile as tile
from concourse import bacc, bass_utils, mybir

F32 = mybir.dt.float32
F32R = mybir.dt.float32r
BF16 = mybir.dt.bfloat16
AF = mybir.ActivationFunctionType
OP = mybir.AluOpType

P = 128
C = 1024
N = 1024
B = 4
H = 16
DH = 64
DFF = 4096
TOK = 512          # per-core query-token block
NCT = C // P       # 8 c-tiles
NFT = DFF // P     # 32 d'-tiles
EPS = 1e-5
SCALE = DH ** -0.5

_CACHE = {}


def build():
    nc = bacc.Bacc(
        "TRN2",
        target_bir_lowering=False,
        debug=False,
        enable_asserts=False,
        num_devices=8,
    )

    def din(name, shape, dt=F32R):
        return nc.dram_tensor(name, shape, dt, kind="ExternalInput").ap()

    xrow = din("xrow", [C, N], BF16)      # rotated x[b].T (bf16), q-window first
    wqkv = din("wqkv", [C, 3 * C], BF16)  # (qkv rows * ln1_g).T
    wproj = din("wproj", [C, C], BF16)    # proj_w.T
    wfc1 = din("wfc1", [C, DFF], BF16)    # (fc1_w * ln2_g).T
    wfc2 = din("wfc2", [DFF, C], BF16)    # (fc2_w * lnh_g).T
    wqs = din("wqs", [P, 2 * NCT], F32)   # per-col rowsums of folded q,k weights
    wvs = din("wvs", [1, C], BF16)        # rowsums for V cols (row layout)
    w1s = din("w1s", [P, NFT], F32)       # per-col rowsums of folded fc1
    w2s = din("w2s", [P, NCT], F32)       # per-col rowsums of folded fc2
    pb = din("pb", [P, NCT], F32)
    f1b = din("f1b", [P, NFT], F32)
    f2b = din("f2b", [P, NCT], F32)
    ident_d = din("ident", [P, P], BF16)  # identity (transposes)

    outT = nc.dram_tensor("outT", [C, TOK], F32, kind="ExternalOutput").ap()

    with tile.TileContext(nc) as tc:
        const = tc.alloc_tile_pool(name="const", bufs=1)
        big = tc.alloc_tile_pool(name="big", bufs=1)
        tmp = tc.alloc_tile_pool(name="tmp", bufs=2)
        misc = tc.alloc_tile_pool(name="misc", bufs=1)
        wpool = tc.alloc_tile_pool(name="w", bufs=5)

        # --- constants ---
        ones1 = const.tile([1, P], BF16)
        nc.vector.memset(ones1[:], 1.0)
        onesc_b = const.tile([P, 1], BF16)
        nc.vector.memset(onesc_b[:], 1.0)
        eps_col = const.tile([P, 1], F32)
        nc.vector.memset(eps_col[:], EPS)
        ident = const.tile([P, P], BF16)
        nc.sync.dma_start(ident[:], ident_d[:])
        gb = {}
        for nm, ap_, w in (("wqs", wqs, 2 * NCT), ("pb", pb, NCT),
                           ("f1b", f1b, NFT), ("f2b", f2b, NCT),
                           ("w1s", w1s, NFT), ("w2s", w2s, NCT)):
            t = const.tile([P, w], F32, name=nm, tag=nm)
            nc.sync.dma_start(t[:], ap_[:])
            gb[nm] = t
        wvs_s = const.tile([1, C], BF16)
        nc.sync.dma_start(wvs_s[:], wvs[:])

        # --- load x (rotated row; query window = cols 0:TOK) ---
        xr = big.tile([P, NCT, N], BF16, tag="A")
        xrs = xrow.rearrange("(i p) t -> p i t", p=P)
        for h_ in range(4):
            nc.sync.dma_start(xr[:, 2 * h_:2 * h_ + 2, :], xrs[:, 2 * h_:2 * h_ + 2, :])

        # ---------- LN stat helpers (column trick) ----------
        def ln_cols_finish(S, nch, n_elems, name):
            """S: psum [P, 2*nch] (cols 0..nch-1 sums, nch..2nch-1 sumsq).
            Returns (rstd_cb, nmr_cb) [P, nch] bf16 column tiles."""
            inv = 1.0 / n_elems
            mu = misc.tile([P, nch], F32R, tag="lnf", bufs=4, name=f"mu_{name}")
            nc.vector.tensor_scalar_mul(mu[:], S[:, 0:nch], inv)
            ex2 = misc.tile([P, nch], F32, tag="lnf", bufs=4, name=f"ex2_{name}")
            nc.vector.tensor_scalar_mul(ex2[:], S[:, nch:2 * nch], inv)
            mu2 = misc.tile([P, nch], F32, tag="lnf", bufs=4, name=f"mu2_{name}")
            nc.vector.tensor_mul(mu2[:], mu[:], mu[:])
            nc.vector.tensor_sub(ex2[:], ex2[:], mu2[:])
            nc.scalar.activation(ex2[:], ex2[:], AF.Ln, bias=eps_col[:])
            rstd = misc.tile([P, nch], F32, tag="lnc", bufs=6, name=f"rstd_{name}")
            nc.scalar.activation(rstd[:], ex2[:], AF.Exp, scale=-0.5)
            nmr = misc.tile([P, nch], F32, tag="lnc", bufs=6, name=f"nmr_{name}")
            nc.vector.scalar_tensor_tensor(nmr[:], mu[:], -1.0, rstd[:],
                                           op0=OP.mult, op1=OP.mult)
            rstd_cb = misc.tile([P, nch], BF16, tag="lncb", bufs=6, name=f"rcb_{name}")
            nc.vector.tensor_copy(rstd_cb[:], rstd[:])
            nmr_cb = misc.tile([P, nch], BF16, tag="lncb", bufs=6, name=f"ncb_{name}")
            nc.vector.tensor_copy(nmr_cb[:], nmr[:])
            return rstd, nmr, rstd_cb, nmr_cb

        def ln_rows_bcast(ps_row, ps_bc, rstd_cb, nmr_cb, nch, name):
            """Columns [P, nch] -> broadcast tiles [P, nch*P] bf16 (rstd_b, nmr_b)."""
            outs = []
            for cb, nm in ((rstd_cb, "r"), (nmr_cb, "n")):
                rowp = ps_row.tile([1, nch * P], BF16, tag="row",
                                   name=f"rp_{name}{nm}")
                for ch in range(nch):
                    nc.tensor.matmul(rowp[0:1, ch * P:(ch + 1) * P],
                                     cb[:, ch:ch + 1], ident[:],
                                     is_transpose=True, start=True, stop=True)
                row = misc.tile([1, nch * P], BF16, tag="lnrow", bufs=2,
                                name=f"row_{name}{nm}")
                nc.vector.tensor_copy(row[:], rowp[0:1, :])
                bcast = misc.tile([P, nch * P], BF16, tag=f"lnb{nch}",
                                  bufs=(2 if nch == 8 else 4), name=f"b_{name}{nm}")
                for hh in range(nch * P // TOK):
                    bp = ps_bc.tile([P, TOK], F32, tag="bc", name=f"bc_{name}{nm}{hh}")
                    nc.tensor.matmul(bp[:], ones1[:], row[0:1, hh * TOK:(hh + 1) * TOK],
                                     start=True, stop=True)
                    nc.vector.tensor_copy(bcast[:, hh * TOK:(hh + 1) * TOK], bp[:])
                outs.append(bcast)
            return outs

        # --- LN1 stats over the full (rotated) row: 8 chunks of 128 tokens ---
        ps_ln1 = tc.alloc_tile_pool(name="ps_ln1", bufs=1, space="PSUM")
        ps_row = tc.alloc_tile_pool(name="ps_row", bufs=2, space="PSUM")
        ps_bc = tc.alloc_tile_pool(name="ps_bc", bufs=2, space="PSUM")
        S1 = ps_ln1.tile([P, 16], F32, tag="S1")
        for ci in range(NCT):
            # squares split across ACT and DVE to halve serial latency
            s = tmp.tile([P, N], BF16, tag="sq1", bufs=2, name=f"sq1_{ci}")
            if ci % 2 == 0:
                nc.scalar.activation(s[:], xr[:, ci, :], AF.Square)
            else:
                nc.vector.tensor_mul(s[:], xr[:, ci, :], xr[:, ci, :])
            for ch in range(NCT):
                nc.tensor.matmul(S1[:, ch:ch + 1], xr[:, ci, ch * P:(ch + 1) * P],
                                 onesc_b[:], start=(ci == 0), stop=(ci == NCT - 1))
                nc.tensor.matmul(S1[:, 8 + ch:9 + ch], s[:, ch * P:(ch + 1) * P],
                                 onesc_b[:], start=(ci == 0), stop=(ci == NCT - 1))
        rstd1_c, nmr1_c, rstd1_cb, nmr1_cb = ln_cols_finish(S1, NCT, C, "ln1")
        rstd1_b, nmr1_b = ln_rows_bcast(ps_row, ps_bc, rstd1_cb, nmr1_cb, NCT, "ln1")

        # wvs broadcast to all partitions (for V eviction outer product)
        wvs_b = misc.tile([P, C], BF16, tag="wvs_b")
        for g in range(2):
            bp = ps_bc.tile([P, TOK], F32, tag="bc", name=f"bcv{g}")
            nc.tensor.matmul(bp[:], ones1[:], wvs_s[0:1, g * TOK:(g + 1) * TOK],
                             start=True, stop=True)
            nc.vector.tensor_copy(wvs_b[:, g * TOK:(g + 1) * TOK], bp[:])
        ps_bc.release()
        ps_row.release()
        ps_ln1.release()

        # --- QKV with fused LN1 ---
        KT = big.tile([P, NCT, N], BF16, tag="B")
        QT = big.tile([P, NCT, TOK], BF16, tag="F")
        V = big.tile([P, NCT, H, DH + 1], BF16, tag="V")
        for r in range(NCT):
            nc.vector.memset(V[:, r, :, DH:DH + 1], 1.0)

        ps_acc = tc.alloc_tile_pool(name="ps_qkv", bufs=2, space="PSUM")
        ps_s = tc.alloc_tile_pool(name="ps_s", bufs=3, space="PSUM")
        wqkv_r = wqkv.rearrange("(i p) n -> p i n", p=P)

        def qkv_wload(g):
            w = wpool.tile([P, NCT, 512], BF16, tag="w", name=f"wg{g}")
            nc.sync.dma_start(w[:], wqkv_r[:, :, g * 512:(g + 1) * 512])
            return w

        def qk_group(g, wt):
            if g < 2:  # Q -> QT (query block only)
                for jt in range(4):
                    jj = g * 4 + jt
                    ps = ps_acc.tile([P, TOK], F32, tag="acc")
                    for ci in range(NCT):
                        nc.tensor.matmul(ps[:], wt[:, ci, jt * P:(jt + 1) * P],
                                         xr[:, ci, 0:TOK],
                                         start=(ci == 0), stop=(ci == NCT - 1))
                    t = tmp.tile([P, TOK], BF16, tag="ev", bufs=3)
                    nc.vector.tensor_mul(t[:], ps[:], rstd1_b[:, 0:TOK])
                    nc.vector.scalar_tensor_tensor(
                        QT[:, jj, :], nmr1_b[:, 0:TOK], gb["wqs"][:, jj:jj + 1], t[:],
                        op0=OP.mult, op1=OP.add)
            else:  # K -> KT (full row)
                for jt in range(4):
                    jj = (g - 2) * 4 + jt
                    for blk in range(2):
                        sl = slice(blk * TOK, (blk + 1) * TOK)
                        ps = ps_acc.tile([P, TOK], F32, tag="acc")
                        for ci in range(NCT):
                            nc.tensor.matmul(ps[:], wt[:, ci, jt * P:(jt + 1) * P],
                                             xr[:, ci, sl],
                                             start=(ci == 0), stop=(ci == NCT - 1))
                        t = tmp.tile([P, TOK], BF16, tag="ev", bufs=3)
                        nc.vector.tensor_mul(t[:], ps[:], rstd1_b[:, sl])
                        nc.vector.scalar_tensor_tensor(
                            KT[:, jj, sl], nmr1_b[:, sl],
                            gb["wqs"][:, NCT + jj:NCT + jj + 1], t[:],
                            op0=OP.mult, op1=OP.add)

        def v_group(g, wt):  # g in (4, 5)
            h0 = 8 * (g - 4)
            dsl = slice((g - 4) * TOK, (g - 4 + 1) * TOK)
            for r in range(NCT):
                ps = ps_acc.tile([P, TOK], F32, tag="acc")
                for ci in range(NCT):
                    nc.tensor.matmul(ps[:], xr[:, ci, r * P:(r + 1) * P],
                                     wt[:, ci, :], start=(ci == 0), stop=(ci == NCT - 1))
                t = tmp.tile([P, TOK], BF16, tag="ev", bufs=3)
                nc.vector.tensor_scalar_mul(t[:], wvs_b[:, dsl], nmr1_c[:, r:r + 1])
                nc.vector.scalar_tensor_tensor(
                    V[:, r, h0:h0 + 8, 0:DH],
                    ps[:].rearrange("p (h d) -> p h d", h=8),
                    rstd1_c[:, r:r + 1],
                    t[:].rearrange("p (h d) -> p h d", h=8),
                    op0=OP.mult, op1=OP.add)

        Es = {}

        def s_exp(jj):
            E_l = []
            for kt in range(NCT):
                E_t = big.tile([P, 2 * TOK], BF16, tag="E", bufs=16,
                               name=f"E{jj}_{kt}")
                E_l.append(E_t)
                ks = slice(kt * P, (kt + 1) * P)
                psa = ps_s.tile([P, TOK], F32, tag="S", name=f"Sa{jj}_{kt}")
                nc.tensor.matmul(psa[:], KT[0:64, jj, ks], QT[0:64, jj, :],
                                 start=True, stop=True, tile_position=(0, 0))
                nc.scalar.activation(E_t[:, 0:TOK], psa[:], AF.Exp, scale=SCALE)
                psb = ps_s.tile([P, TOK], F32, tag="S", name=f"Sb{jj}_{kt}")
                nc.tensor.matmul(psb[:], KT[64:128, jj, ks], QT[64:128, jj, :],
                                 start=True, stop=True, tile_position=(64, 0))
                nc.scalar.activation(E_t[:, TOK:2 * TOK], psb[:], AF.Exp, scale=SCALE)
            Es[jj] = E_l

        def av(jj):
            """Transposed A@V: E chunks stationary, [V|1] moving (N=65)."""
            E_l = Es.pop(jj)
            OTT = misc.tile([P, 4, P], BF16, tag="OTT", bufs=3, name=f"OTT{jj}")
            for qc in range(4):
                pO = [ps_o.tile([P, DH + 1], F32, tag="O", name=f"pO{jj}_{qc}_{h2}")
                      for h2 in range(2)]
                for h2 in range(2):
                    h = 2 * jj + h2
                    for kt in range(NCT):
                        nc.tensor.matmul(pO[h2][:],
                                         E_l[kt][:, h2 * TOK + qc * P:
                                                  h2 * TOK + (qc + 1) * P],
                                         V[:, kt, h, :],
                                         start=(kt == 0), stop=(kt == NCT - 1))
                rec = misc.tile([P, 2], F32, tag="rec", bufs=3, name=f"rec{jj}_{qc}")
                for h2 in range(2):
                    with nc.allow_low_precision(reason="softmax denom"):
                        nc.vector.reciprocal(rec[:, h2:h2 + 1], pO[h2][:, DH:DH + 1])
                    nc.vector.tensor_scalar_mul(
                        OTT[:, qc, h2 * DH:(h2 + 1) * DH], pO[h2][:, 0:DH],
                        rec[:, h2:h2 + 1])
                tp = ps_o.tile([P, P], BF16, tag="tp", bufs=1, name=f"tp{jj}_{qc}")
                nc.tensor.matmul(tp[:], OTT[:, qc, :], ident[:],
                                 is_transpose=True, start=True, stop=True)
                nc.vector.tensor_copy(OT[:, jj, qc * P:(qc + 1) * P], tp[:])

        OT = big.tile([P, NCT, TOK], BF16, tag="C")
        ps_o = tc.alloc_tile_pool(name="ps_o", bufs=2, space="PSUM")

        wts = {g: qkv_wload(g) for g in range(2)}
        for g in range(6):
            if g + 2 < 6:
                wts[g + 2] = qkv_wload(g + 2)
            if g < 4:
                qk_group(g, wts.pop(g))
            else:
                v_group(g, wts.pop(g))
        s_exp(0)
        s_exp(1)
        for jj in range(NCT):
            av(jj)
            if jj + 2 < NCT:
                s_exp(jj + 2)
        for p_ in (ps_o, ps_s, ps_acc):
            p_.release()

        # --- output projection + residual -> x2 (f32r) & x2b (bf16) ---
        x2 = big.tile([P, NCT, TOK], F32R, tag="D")
        x2b = big.tile([P, NCT, TOK], BF16, tag="F")  # reuses QT slot
        ps_ln2 = tc.alloc_tile_pool(name="ps_ln2", bufs=1, space="PSUM")
        S2 = ps_ln2.tile([P, 8], F32, tag="S2")
        ps_acc = tc.alloc_tile_pool(name="ps_proj", bufs=4, space="PSUM")
        wproj_r = wproj.rearrange("(i p) n -> p i n", p=P)
        for ig in range(2):
            w = wpool.tile([P, NCT, 512], BF16, tag="w", name=f"wp{ig}")
            nc.sync.dma_start(w[:], wproj_r[:, :, ig * 512:(ig + 1) * 512])
            for i4 in range(4):
                i = ig * 4 + i4
                ps = ps_acc.tile([P, TOK], F32, tag="acc")
                for ci in range(NCT):
                    nc.tensor.matmul(ps[:], w[:, ci, i4 * P:(i4 + 1) * P],
                                     OT[:, ci, :], start=(ci == 0), stop=(ci == NCT - 1))
                nc.vector.scalar_tensor_tensor(
                    x2[:, i, :], ps[:], gb["pb"][:, i:i + 1], xr[:, i, 0:TOK],
                    op0=OP.add, op1=OP.add)
                nc.scalar.activation(x2b[:, i, :], x2[:, i, :], AF.Copy)
                sq = tmp.tile([P, TOK], BF16, tag="ln_sq")
                nc.vector.tensor_mul(sq[:], x2[:, i, :], x2[:, i, :])
                for ch in range(4):
                    nc.tensor.matmul(S2[:, ch:ch + 1],
                                     x2b[:, i, ch * P:(ch + 1) * P], onesc_b[:],
                                     start=(i == 0), stop=(i == NCT - 1))
                    nc.tensor.matmul(S2[:, 4 + ch:5 + ch],
                                     sq[:, ch * P:(ch + 1) * P], onesc_b[:],
                                     start=(i == 0), stop=(i == NCT - 1))
        ps_acc.release()

        _, _, rstd2_cb, nmr2_cb = ln_cols_finish(S2, 4, C, "ln2")
        ps_row = tc.alloc_tile_pool(name="ps_row2", bufs=2, space="PSUM")
        ps_bc = tc.alloc_tile_pool(name="ps_bc2", bufs=2, space="PSUM")
        rstd2_b, nmr2_b = ln_rows_bcast(ps_row, ps_bc, rstd2_cb, nmr2_cb, 4, "ln2")
        ps_bc.release()
        ps_row.release()
        ps_ln2.release()

        # --- fc1 (LN2 folded into eviction) + LNh stats + fc2 first half ---
        U0 = big.tile([P, NFT // 2, TOK], BF16, tag="B")   # reuses KT slot
        U1 = big.tile([P, NFT // 2, TOK], BF16, tag="V")   # reuses V slot

        def u_tile(i):
            return (U0 if i < NFT // 2 else U1)[:, i % (NFT // 2), :]

        ps_f2 = tc.alloc_tile_pool(name="ps_fc2", bufs=1, space="PSUM")
        fp2 = [ps_f2.tile([P, TOK], F32, tag=f"f2_{j}", name=f"f2_{j}")
               for j in range(4)]
        ps_lnh = tc.alloc_tile_pool(name="ps_lnh", bufs=1, space="PSUM")
        Sh = ps_lnh.tile([P, 8], F32, tag="Sh")
        ps_f1 = tc.alloc_tile_pool(name="ps_fc1", bufs=2, space="PSUM")
        wfc1_r = wfc1.rearrange("(i p) n -> p i n", p=P)
        wfc2_r = wfc2.rearrange("(i p) n -> p i n", p=P)
        w1t = {}
        w2t = {}
        sqh = {}

        def fc1_step(i):
            ig, i4 = divmod(i, 4)
            if i4 == 0:
                w = wpool.tile([P, NCT, 512], BF16, tag="w", name=f"w1_{ig}")
                nc.sync.dma_start(w[:], wfc1_r[:, :, ig * 512:(ig + 1) * 512])
                w1t[ig] = w
                if ig > 0:
                    del w1t[ig - 1]
            if i % 8 == 0:
                cc = i // 8
                w = wpool.tile([P, NCT, 512], BF16, tag="w", name=f"w2a_{cc}")
                nc.sync.dma_start(w[:], wfc2_r[:, cc * 8:cc * 8 + 8, 0:512])
                w2t[cc] = w
            w = w1t[i // 4]
            ps = ps_f1.tile([P, TOK], F32, tag="acc")
            for ci in range(NCT):
                nc.tensor.matmul(ps[:], w[:, ci, (i % 4) * P:(i % 4 + 1) * P],
                                 x2b[:, ci, :], start=(ci == 0), stop=(ci == NCT - 1))
            t = tmp.tile([P, TOK], F32R, tag="ev1", bufs=3)
            nc.vector.tensor_mul(t[:], ps[:], rstd2_b[:])
            t2 = tmp.tile([P, TOK], F32R, tag="ev2", bufs=3)
            nc.vector.scalar_tensor_tensor(t2[:], nmr2_b[:], gb["w1s"][:, i:i + 1],
                                           t[:], op0=OP.mult, op1=OP.add)
            nc.scalar.activation(u_tile(i), t2[:], AF.Gelu, bias=gb["f1b"][:, i:i + 1])
            s = tmp.tile([P, TOK], BF16, tag="sqh", bufs=3, name=f"sqh{i}")
            nc.scalar.activation(s[:], u_tile(i), AF.Square)
            sqh[i] = s

        def hstats_and_fc2a(i):
            s = sqh.pop(i)
            for ch in range(4):
                nc.tensor.matmul(Sh[:, ch:ch + 1],
                                 u_tile(i)[:, ch * P:(ch + 1) * P], onesc_b[:],
                                 start=(i == 0), stop=(i == NFT - 1))
                nc.tensor.matmul(Sh[:, 4 + ch:5 + ch],
                                 s[:, ch * P:(ch + 1) * P], onesc_b[:],
                                 start=(i == 0), stop=(i == NFT - 1))
            w = w2t[i // 8]
            for j in range(4):
                nc.tensor.matmul(fp2[j][:], w[:, i % 8, j * P:(j + 1) * P],
                                 u_tile(i), start=(i == 0), stop=(i == NFT - 1))

        for i in range(NFT):
            fc1_step(i)
            if i >= 1:
                hstats_and_fc2a(i - 1)
        hstats_and_fc2a(NFT - 1)
        ps_f1.release()

        _, _, rstdh_cb, nmrh_cb = ln_cols_finish(Sh, 4, DFF, "lnh")
        ps_lnh.release()
        ps_row = tc.alloc_tile_pool(name="ps_rowh", bufs=1, space="PSUM")
        ps_bc = tc.alloc_tile_pool(name="ps_bch", bufs=1, space="PSUM")
        rstdh_b, nmrh_b = ln_rows_bcast(ps_row, ps_bc, rstdh_cb, nmrh_cb, 4, "lnh")
        ps_bc.release()
        ps_row.release()

        OS = big.tile([P, NCT, TOK], F32, tag="A")  # reuses xr slot

        def fc2_evict(j, fps):
            t = tmp.tile([P, TOK], F32R, tag="ev1", bufs=3)
            nc.vector.tensor_mul(t[:], fps[:], rstdh_b[:])
            t2 = tmp.tile([P, TOK], F32R, tag="ev2", bufs=3)
            nc.vector.scalar_tensor_tensor(t2[:], nmrh_b[:], gb["w2s"][:, j:j + 1],
                                           t[:], op0=OP.mult, op1=OP.add)
            nc.vector.scalar_tensor_tensor(OS[:, j, :], t2[:], gb["f2b"][:, j:j + 1],
                                           x2[:, j, :], op0=OP.add, op1=OP.add)

        # --- fc2 second half streams; first-half evictions overlap it ---
        ps_f2b = tc.alloc_tile_pool(name="ps_fc2b", bufs=1, space="PSUM")
        fp2b = [ps_f2b.tile([P, TOK], F32, tag=f"f2b_{j}", name=f"f2b_{j}")
                for j in range(4)]
        outT_r = outT.rearrange("(i p) t -> p i t", p=P)
        for i in range(NFT):
            if i % 8 == 0:
                cc = i // 8
                w = wpool.tile([P, NCT, 512], BF16, tag="w", name=f"w2b_{cc}")
                nc.sync.dma_start(w[:], wfc2_r[:, cc * 8:cc * 8 + 8, 512:1024])
                w2t[4 + cc] = w
            w = w2t[4 + i // 8]
            for j in range(4):
                nc.tensor.matmul(fp2b[j][:], w[:, i % 8, j * P:(j + 1) * P],
                                 u_tile(i), start=(i == 0), stop=(i == NFT - 1))
            if i == 4:
                for j in range(4):
                    fc2_evict(j, fp2[j])
                nc.sync.dma_start(outT_r[:, 0:4, :], OS[:, 0:4, :])
        for j in range(4):
            fc2_evict(4 + j, fp2b[j])
        nc.sync.dma_start(outT_r[:, 4:8, :], OS[:, 4:8, :])
        ps_f2b.release()
        ps_f2.release()

        for p_ in (wpool, misc, tmp, big, const):
            p_.release()

    nc.compile()
    return nc


def _prep_inputs(inputs):
    """Host-side transposes/folds/rotations -> per-core in_maps."""
    f = lambda a: np.asarray(a, dtype=np.float32)
    x = f(inputs["x"])
    xT = np.ascontiguousarray(x.transpose(0, 2, 1))          # [B, C, N]

    g1 = f(inputs["ln1_g"])
    g2 = f(inputs["ln2_g"])
    ghv = f(inputs["lnh_g"])
    for nm in ("ln1_b", "ln2_b", "lnh_b"):
        if np.abs(f(inputs[nm])).max() != 0.0:
            raise NotImplementedError(f"{nm} != 0 not supported by this kernel")

    qkv_f = f(inputs["qkv_w"]) * g1[None, :]      # fold ln1_g
    fc1_f = f(inputs["fc1_w"]) * g2[None, :]      # fold ln2_g
    fc2_f = f(inputs["fc2_w"]) * ghv[None, :]     # fold lnh_g
    qs = qkv_f.sum(axis=1)                        # [3072] rowsums

    bf = ml_dtypes.bfloat16
    common = {
        "wqkv": np.ascontiguousarray(qkv_f.T.astype(bf)),
        "wproj": np.ascontiguousarray(f(inputs["proj_w"]).T.astype(bf)),
        "wfc1": np.ascontiguousarray(fc1_f.T.astype(bf)),
        "wfc2": np.ascontiguousarray(fc2_f.T.astype(bf)),
        "wqs": np.ascontiguousarray(qs[:2 * C].reshape(2 * NCT, P).T),
        "wvs": np.ascontiguousarray(qs[2 * C:].reshape(1, C).astype(bf)),
        "w1s": np.ascontiguousarray(fc1_f.sum(axis=1).reshape(NFT, P).T),
        "w2s": np.ascontiguousarray(fc2_f.sum(axis=1).reshape(NCT, P).T),
        "pb": np.ascontiguousarray(f(inputs["proj_b"]).reshape(NCT, P).T),
        "f1b": np.ascontiguousarray(f(inputs["fc1_b"]).reshape(NFT, P).T),
        "f2b": np.ascontiguousarray(f(inputs["fc2_b"]).reshape(NCT, P).T),
        "ident": np.eye(P, dtype=bf),
    }
    in_maps = []
    for c in range(8):
        b, off = c // 2, (c % 2) * TOK
        m = dict(common)
        xb = xT[b].astype(bf)
        m["xrow"] = np.ascontiguousarray(
            np.concatenate([xb[:, off:off + TOK], xb[:, TOK - off:N - off]], axis=1))
        in_maps.append(m)
    return in_maps


def _assemble(results):
    out = np.empty((B, N, C), np.float32)
    for c in range(8):
        b, off = c // 2, (c % 2) * TOK
        out[b, off:off + TOK, :] = results[c]["outT"].T
    return out


def kernel(**inputs) -> np.ndarray:
    nc = _CACHE.get("nc")
    if nc is None:
        nc = build()
        _CACHE["nc"] = nc
    in_maps = _prep_inputs(inputs)
    res = bass_utils.run_bass_kernel_spmd(nc, in_maps, core_ids=list(range(8)))
    return _assemble(res.results)
